# revision 17
# baseline (speedup 1.0000x reference)
"""Trainium2 Bass kernel for nn_Decorrelation (cubic B-spline decorrelation).

Math: out[n,v] = x[n,v] + sum_{c<v} (B(x_c) @ params[:, pair(v,c)]) * x[n,c]
where B is the cubic B-spline basis (11 funcs, uniform knots) of the clamped
input, plus three scalar ridge penalties computed from params alone.

Device strategy (pure data parallel over 8 cores, rows sharded):
  - rows-on-partition layout; groups of 2048 rows (16 blocks of 128);
    partition p holds rows [16p, 16p+16) of its group so DMA reads are
    1KB-contiguous per partition (row order within a block is permuted,
    which is harmless: every stage uses the same access pattern).
  - compact: u = affine(clamp(x))  (2 tensor_scalar ops)
  - expanded (128, 16*176): slot (c,k) holds 6*b3_k(u_c)*x_c where
      b3(t) = (relu(m)^3 - 4*relu(m-1)^3)/6,  m = 2 - |t - 2|,  t = u - k + 3
    computed with standard ops (custom DVE ops are broken in this walrus):
      d = u - (k-1) [GPSIMD]; aa = |d| [ACT]; mb1 = min(aa,2)-2, mb2 =
      min(aa,1)-1 [DVE]; s1 = mb1^2, s2 = (2*mb2)^2 [ACT Square];
      c1 = s1*mb1 = -r1^3, c2 = s2*mb2 = -4*r2^3 [DVE];
      num = c2 - c1 = 6*b3 [GPSIMD]; g = num * x [DVE] -> bf16
    with group-skewed emission (5-stage software pipeline) so the in-order
    engines overlap across groups.
  - PE-transpose g blocks, K=176 (=128+48) bf16 matmul against
    host-precomputed W[(c,k),v] = params[k, pair(v,c)]/6
  - PE-transpose the (16, n) correction back to natural, add x, store.
"""

import sys
import numpy as np

if "/opt/trn_rl_repo" not in sys.path:
    sys.path.insert(0, "/opt/trn_rl_repo")

import ml_dtypes  # noqa: E402
import concourse.bass as bass  # noqa: E402
import concourse.mybir as mybir  # noqa: E402
from concourse.tile import TileContext  # noqa: E402
from concourse.vector_clock import ScopedClock  # noqa: E402
from concourse.bass_utils import run_bass_kernel_spmd  # noqa: E402

N_FULL, D, KB = 262144, 16, 11  # rows, dims, basis funcs
NCORES = 8
R = N_FULL // NCORES  # 32768 rows per core
GROUP_ROWS = 1024  # 8 blocks of 128 rows
TPB = GROUP_ROWS // 128  # 16 rows per partition per group
NGROUPS = R // GROUP_ROWS  # 16
NSLOT = D * KB  # 176
EXPW = TPB * NSLOT  # 2816

F32 = mybir.dt.float32
BF16 = mybir.dt.bfloat16
Alu = mybir.AluOpType
Actf = mybir.ActivationFunctionType


# --------------------------------------------------------------------------
# workaround: walrus in this container rejects ANY instruction with >1 sync
# wait. Split multi-wait instructions into single-wait same-engine nops.
# --------------------------------------------------------------------------
def _split_multi_waits(nc, ordered_by_block):
    for bb_name, lst in ordered_by_block.items():
        new = []
        for inst in lst:
            si = getattr(inst, "sync_info", None)
            waits = list(si.on_wait) if (si is not None and si.on_wait) else []
            if len(waits) > 1:
                for w in waits[:-1]:
                    nop = mybir.InstNoOp(
                        name=nc.get_next_instruction_name(), ins=[], outs=[]
                    )
                    nop.engine = inst.engine
                    nop.sync_info = mybir.SyncInfo(on_wait=[w], on_update=[])
                    new.append(nop)
                inst.sync_info = mybir.SyncInfo(
                    on_wait=waits[-1:], on_update=list(si.on_update or [])
                )
            new.append(inst)
        lst[:] = new


def _patch_tile_drain():
    if getattr(TileContext, "_drain_patched", False):
        return

    import concourse.tile as tile_mod

    _Real = tile_mod.TileClockWait

    class _SplitTileClockWait:
        def __init__(self, tc, ordered_instructions_by_block, **kw):
            self._real = _Real(tc, ordered_instructions_by_block, **kw)
            self._ordered = ordered_instructions_by_block
            self._nc = tc.nc

        def assign_waits(self, bb_name):
            r = self._real.assign_waits(bb_name)
            _split_multi_waits(self._nc, self._ordered)
            return r

        def __getattr__(self, k):
            return getattr(self._real, k)

    tile_mod.TileClockWait = _SplitTileClockWait

    def _drain_and_barrier_split(self, tick_clock, wait_clock):
        probe = self.nc.sync.nop(nofuse=True, hint="tail_wait_probe")
        wait_clock.add_sem_waits(
            probe.ins, ScopedClock({None: tick_clock.global_clock})
        )
        si = probe.ins.sync_info
        waits = list(si.on_wait) if (si is not None and si.on_wait) else []
        if len(waits) > 1:
            probe.ins.sync_info = mybir.SyncInfo(
                on_wait=waits[:1], on_update=list(si.on_update or [])
            )
            for i in range(1, len(waits)):
                extra = self.nc.sync.nop(nofuse=True, hint=f"tail_wait_{i}")
                extra.ins.sync_info = mybir.SyncInfo(
                    on_wait=waits[i : i + 1], on_update=[]
                )
        self.nc.sync.drain()
        self.nc.all_engine_barrier()
        assert self.sems is not None
        popped = self.nc._tile_sem_poison_stack.pop()
        assert popped is self._sem_poison
        self.nc.clear_and_free_semaphores(list(self.sems.allocated().values()))
        self.nc.all_engine_barrier()

    TileContext._drain_and_barrier = _drain_and_barrier_split
    TileContext._drain_patched = True


# --------------------------------------------------------------------------
# custom DVE op: out = (min(|in0 - in1|, s0) - s0)^3 * s1
# --------------------------------------------------------------------------
_SPLINE_CUBE = None


def _register_spline_cube():
    global _SPLINE_CUBE
    if _SPLINE_CUBE is not None:
        return _SPLINE_CUBE
    import concourse.dve_ops as dve_ops_mod
    from concourse.dve_ops import DveOp
    from concourse.dve_spec import (
        C0,
        C1,
        AluOp,
        Bin,
        Spec,
        Src0,
        Src1,
        Zero,
        lower,
        maxx,
        minn,
    )
    from concourse.dve_uop import DveOpSpec

    d = Bin(AluOp.SUBTRACT, Src0, Src1)
    nd = Bin(AluOp.SUBTRACT, Zero, d)
    a = maxx(d, nd)
    m = minn(a, C0)
    mb = Bin(AluOp.SUBTRACT, m, C0)
    q = Bin(AluOp.MULTIPLY, mb, mb)
    c = Bin(AluOp.MULTIPLY, q, mb)
    body = Bin(AluOp.MULTIPLY, c, C1)

    def ref(in0, in1, s0, s1, imm2):
        aa = np.abs(in0.astype(np.float32) - in1.astype(np.float32))
        mbb = np.minimum(aa, np.float32(s0)) - np.float32(s0)
        return (mbb ** 3 * np.float32(s1)).astype(np.float32)

    spec = Spec(body=body, reference=ref)

    mul_body = Bin(AluOp.MULTIPLY, Bin(AluOp.MULTIPLY, Src0, Src1), C0)

    def mul_ref(in0, in1, s0, s1, imm2):
        return (
            in0.astype(np.float32) * in1.astype(np.float32) * np.float32(s0)
        ).astype(np.float32)

    mul_spec = Spec(body=mul_body, reference=mul_ref)

    def _reg(name, sp):
        shas = {}
        for ver in ("v3",):
            uops = lower(sp, ver=ver)
            shas[ver] = DveOpSpec(
                name=name, opcode=0, uops=uops, rd1_en=True
            ).sha(ver)
        op = DveOp(name, sp, subdim=False, uops_sha=shas)
        if not any(o.name == op.name for o in dve_ops_mod.OPS):
            dve_ops_mod.OPS.append(op)
            dve_ops_mod.CUSTOM_DVE_SPECS[op.name] = op.spec
            dve_ops_mod._SUB_OPCODE_FOR_NAME[op.name] = (
                dve_ops_mod._CUSTOM_DVE_ROW_BASE + len(dve_ops_mod.OPS) - 1
            )
            assert dve_ops_mod._SUB_OPCODE_FOR_NAME[op.name] < 0x20
        return op

    cube = _reg("SPLINE_CUBE_SCALED", spec)
    mul = _reg("SPLINE_MUL_SCALED", mul_spec)
    _SPLINE_CUBE = (cube, mul)
    return _SPLINE_CUBE


def _build_program(hi_clip, lo_clip, inv_h, ubias):
    """Build the per-core Bass program (SPMD; same program on all cores)."""
    _patch_tile_drain()
    spline_cube, spline_mul = _register_spline_cube()
    nc = bass.Bass(trn_type="TRN2")
    x_d = nc.declare_dram_parameter("x", [R, D], F32, isOutput=False)
    w1_d = nc.declare_dram_parameter("w1", [128, D], BF16, isOutput=False)
    w2_d = nc.declare_dram_parameter("w2", [48, D], BF16, isOutput=False)
    k11_d = nc.declare_dram_parameter("k11", [128, KB], F32, isOutput=False)
    idb_d = nc.declare_dram_parameter("idb", [128, 128], BF16, isOutput=False)
    idf_d = nc.declare_dram_parameter("idf", [16, 16], F32, isOutput=False)
    out_d = nc.declare_dram_parameter("out", [R, D], F32, isOutput=True)

    with TileContext(nc) as tc:
        with (
            tc.tile_pool(name="const", bufs=1) as cpool,
            tc.tile_pool(name="work", bufs=4) as wpool,
            tc.tile_pool(name="exp", bufs=4) as epool,
            tc.tile_pool(name="ps", bufs=2, space="PSUM") as ppool,
        ):
            w1 = cpool.tile([128, D], BF16)
            w2 = cpool.tile([48, D], BF16)
            k11 = cpool.tile([128, KB], F32)
            idb = cpool.tile([128, 128], BF16)
            idf = cpool.tile([16, 16], F32)
            nc.sync.dma_start(out=w1[:], in_=w1_d[:])
            nc.sync.dma_start(out=w2[:], in_=w2_d[:])
            nc.sync.dma_start(out=k11[:], in_=k11_d[:])
            nc.sync.dma_start(out=idb[:], in_=idb_d[:])
            nc.sync.dma_start(out=idf[:], in_=idf_d[:])

            def group_stages(grp):
                base = grp * GROUP_ROWS
                xb = wpool.tile([128, GROUP_ROWS // 8], F32, tag="xb")
                nc.sync.dma_start(
                    out=xb[:].rearrange("p (t c) -> p t c", t=TPB),
                    in_=x_d[base : base + GROUP_ROWS, :].rearrange(
                        "(p t) c -> p t c", p=128
                    ),
                )
                u = wpool.tile([128, GROUP_ROWS // 8], F32, tag="u")
                nc.vector.tensor_scalar(
                    u[:], xb[:], hi_clip, lo_clip, Alu.min, Alu.max
                )
                nc.vector.tensor_scalar(
                    u[:], u[:], inv_h, ubias, Alu.mult, Alu.add
                )
                ubc = u[:][:, :, None].broadcast_to([128, TPB * D, KB])
                xbc = xb[:][:, :, None].broadcast_to([128, TPB * D, KB])
                kbc = k11[:][:, None, :].broadcast_to([128, TPB * D, KB])
                d = epool.tile([128, EXPW], F32, tag="d")
                d3 = d[:].rearrange("p (m k) -> p m k", k=KB)
                nc.gpsimd.tensor_tensor(d3, ubc, kbc, Alu.subtract)
                yield
                nc.scalar.activation(d[:], d[:], Actf.Abs)  # aa in-place
                mb1 = epool.tile([128, EXPW], F32, tag="mb1")
                nc.vector.tensor_scalar(
                    mb1[:], d[:], 2.0, 2.0, Alu.min, Alu.subtract
                )
                mb2 = epool.tile([128, EXPW], F32, tag="mb2")
                nc.vector.tensor_scalar(
                    mb2[:], d[:], 1.0, 1.0, Alu.min, Alu.subtract
                )
                s1 = epool.tile([128, EXPW], F32, tag="s1")
                nc.scalar.activation(s1[:], mb1[:], Actf.Square)
                s2 = epool.tile([128, EXPW], F32, tag="s2")
                nc.scalar.activation(s2[:], mb2[:], Actf.Square, scale=2.0)
                yield
                # c1 = s1*mb1 = -r1^3 (into s1); c2 = s2*mb2 = -4*r2^3 (into s2)
                nc.vector.tensor_tensor(s1[:], s1[:], mb1[:], Alu.mult)
                nc.vector.tensor_tensor(s2[:], s2[:], mb2[:], Alu.mult)
                # num = c2 - c1 = 6*b3 (into mb1, dead)
                eng = nc.gpsimd
                eng.tensor_tensor(mb1[:], s2[:], s1[:], Alu.subtract)
                g = epool.tile([128, EXPW], BF16, tag="g")
                g3 = g[:].rearrange("p (m k) -> p m k", k=KB)
                mb13 = mb1[:].rearrange("p (m k) -> p m k", k=KB)
                nc.vector.tensor_tensor(g3, mb13, xbc, Alu.mult)
                yield
                pnat = ppool.tile([128, GROUP_ROWS // 8], F32, tag="nat")
                cts = []
                for quad in range(TPB // 4):
                    T1 = ppool.tile([128, 512], BF16, tag="T1")
                    T2 = ppool.tile([48, 512], BF16, tag="T2")
                    for j in range(4):
                        ti = 4 * quad + j
                        nc.tensor.matmul(
                            T1[:, 128 * j : 128 * (j + 1)],
                            g[:, NSLOT * ti : NSLOT * ti + 128],
                            idb[:],
                            is_transpose=True,
                        )
                        nc.tensor.matmul(
                            T2[:, 128 * j : 128 * (j + 1)],
                            g[:, NSLOT * ti + 128 : NSLOT * (ti + 1)],
                            idb[:],
                            is_transpose=True,
                        )
                    rhs1 = wpool.tile([128, 512], BF16, tag="rhs1")
                    nc.vector.tensor_copy(rhs1[:], T1[:])
                    rhs2 = wpool.tile([48, 512], BF16, tag="rhs2")
                    nc.scalar.copy(rhs2[:], T2[:])
                    pc = ppool.tile([16, 512], F32, tag="pc")
                    nc.tensor.matmul(pc[:], w1[:], rhs1[:], start=True, stop=False)
                    nc.tensor.matmul(pc[:], w2[:], rhs2[:], start=False, stop=True)
                    ct = wpool.tile([16, 512], F32, tag="ct")
                    nc.scalar.copy(ct[:], pc[:])
                    cts.append(ct)
                yield
                for quad in range(TPB // 4):
                    for j in range(4):
                        ti = 4 * quad + j
                        nc.tensor.matmul(
                            pnat[:, D * ti : D * (ti + 1)],
                            cts[quad][:, 128 * j : 128 * (j + 1)],
                            idf[:],
                            is_transpose=True,
                        )
                outsb = wpool.tile([128, GROUP_ROWS // 8], F32, tag="o")
                nc.vector.tensor_tensor(outsb[:], pnat[:], xb[:], Alu.add)
                nc.sync.dma_start(
                    out=out_d[base : base + GROUP_ROWS, :].rearrange(
                        "(p t) c -> p t c", p=128
                    ),
                    in_=outsb[:].rearrange("p (t c) -> p t c", t=TPB),
                )

            NSTAGE = 5
            gens = [group_stages(g) for g in range(NGROUPS)]
            for step in range(NGROUPS + NSTAGE):
                for off in range(NSTAGE - 1, -1, -1):
                    gg = step - off
                    if 0 <= gg < NGROUPS:
                        next(gens[gg], None)
    return nc


_PROGRAM_CACHE = {}


def _get_program(hi_clip, lo_clip, inv_h, ubias):
    key = (round(float(hi_clip), 9), round(float(lo_clip), 9),
           round(float(inv_h), 9), round(float(ubias), 9))
    if key not in _PROGRAM_CACHE:
        _PROGRAM_CACHE[key] = _build_program(hi_clip, lo_clip, inv_h, ubias)
    return _PROGRAM_CACHE[key]


def _host_prep(params, polynomial_range):
    P = np.asarray(params, np.float32)
    pr = np.asarray(polynomial_range, np.float32)
    K = P.shape[0]  # 11
    low0, high0 = pr[0], pr[1]
    span = high0 - low0
    low = low0 - 0.1 * span
    high = high0 + 0.1 * span
    assert np.allclose(low, low[0]) and np.allclose(high, high[0]), (
        "kernel assumes a uniform polynomial_range across dims"
    )
    lowf = float(low[0])
    highf = float(high[0])
    n_seg = K - 3
    h = (highf - lowf) / n_seg
    eps = 1e-6 * (highf - lowf)
    consts = dict(
        hi_clip=float(np.float32(highf) - np.float32(eps)),
        lo_clip=float(np.float32(lowf)),
        inv_h=float(1.0 / h),
        ubias=float(-lowf / h),
    )
    # weights: W[(c*11+k), v] = params[k, pair(v,c)]/6, pair = v(v-1)/2 + c
    W = np.zeros((NSLOT, D), np.float32)
    for v in range(D):
        for c in range(v):
            l = v * (v - 1) // 2 + c
            W[c * KB : (c + 1) * KB, v] = P[:, l] / 6.0
    Wb = W.astype(ml_dtypes.bfloat16)
    k11 = np.repeat(
        (np.arange(KB, dtype=np.float32) - 1.0)[None, :], 128, axis=0
    )
    idb = np.eye(128, dtype=np.float32).astype(ml_dtypes.bfloat16)
    idf = np.eye(16, dtype=np.float32)
    return consts, Wb, k11, idb, idf


def _make_in_maps(x, Wb, k11, idb, idf):
    return [
        {
            "x": x[i * R : (i + 1) * R],
            "w1": Wb[:128],
            "w2": Wb[128:],
            "k11": k11,
            "idb": idb,
            "idf": idf,
        }
        for i in range(NCORES)
    ]


def kernel(input, params, polynomial_range):
    x = np.ascontiguousarray(np.asarray(input, np.float32))
    P = np.asarray(params, np.float32)
    consts, Wb, k11, idb, idf = _host_prep(P, polynomial_range)
    nc = _get_program(**consts)
    res = run_bass_kernel_spmd(
        nc, _make_in_maps(x, Wb, k11, idb, idf), core_ids=list(range(NCORES))
    )
    out = np.concatenate([res.results[i]["out"] for i in range(NCORES)], axis=0)

    # penalties from params alone (host, fp64 accumulate then fp32 cast)
    P64 = P.astype(np.float64)
    d1 = P64[1:] - P64[:-1]
    d2 = P64[2:] - 2.0 * P64[1:-1] + P64[:-2]
    second_order_pen = np.float32((d2 ** 2).sum())
    first_order_pen = np.float32((d1 ** 2).sum())
    param_pen = np.float32((P64 ** 2).sum())
    return out, second_order_pen, first_order_pen, param_pen
